# revision 10
# baseline (speedup 1.0000x reference)
"""Expert-parallel MoE FFN kernel for 8 trn2 NeuronCores.

Problem (per full input):
  x [4, 8, 512, 1024], audio_feat [4, 256, 1024],
  W1/Wa [8, 1024, 4096], b1 [8, 4096], W2 [8, 4096, 1024], b2 [8, 1024]
  out[b,e,n,:] = gelu_tanh(x[b,e,n] @ W1[e] + b1[e] + mean(audio_feat[b]) @ Wa[e]) @ W2[e] + b2[e]

Sharding: expert-parallel — core e owns expert e (weights + x[:, e] slice);
audio_feat replicated. No collectives needed: shard/gather on host.

Per-core kernel. Main GEMMs in bf16 (same 1 col/cycle PE rate as fp32r but
~11 ns/matmul faster in practice, half the DMA bytes, and FWL halves the
weight-load); audio path in fp8e4 with DoubleRow perf mode (2 fp8 weights
per PE cell -> half the streaming cycles; audio_h is ~6% of h's magnitude
so fp8 noise is negligible). PSUM accumulation is always fp32.
  - audio pooling via DoubleRow matmul with a block-indicator matrix
  - GEMM1 produces h^T tiles [dff, tok] so GEMM2 can consume them as
    stationary operands without transposes (x passed pre-transposed)
  - dff in 8 blocks of 512; GEMM2 partials accumulate into an SBUF
    accumulator via DVE adds; tokens in 2 halves of 1024.
  - All weight/x tensors are host-packed so every DMA is one contiguous
    run per partition (128 descriptors; ~0.6us HWDGE issue instead of
    3-5us for the strided equivalents).
  - DMA ordering: weights + audio on the sync HWDGE queue in exact
    consumption order (FIFO self-throttles the prefetch); x on the scalar
    queue; output tiles on sync (idle by then). Startup-critical bytes
    (af, first w1/wa/w2 blocks) therefore land first.
  - PE warm-up: ~10 free-dim-512 matmuls over a memset tile warm the HAM
    clock gate to 8/8 while af lands, so the real stream never runs cold.
"""
from contextlib import ExitStack

import ml_dtypes
import numpy as np

import concourse.bass as bass
import concourse.tile as tile
from concourse import bacc, mybir
from concourse.bass_utils import run_bass_kernel_spmd

F32 = mybir.dt.float32
BF16 = mybir.dt.bfloat16
FP8 = mybir.dt.float8e4
AF = mybir.ActivationFunctionType
DR = mybir.MatmulPerfMode.DoubleRow

B, E, N, D = 4, 8, 512, 1024
DFF = 4 * D
NA = 256
TOK = B * N            # 2048 tokens per expert
KC = D // 128          # 8 d-chunks
NHALF = 2              # token halves
TOKH = TOK // NHALF    # 1024
NDFB = 8               # dff blocks
DFB = DFF // NDFB      # 512
NC_CORES = 8

_cache = {}


def _build():
    nc = bacc.Bacc("TRN2", target_bir_lowering=False, debug=False,
                   num_devices=NC_CORES)

    # host-packed layouts: leading dims select the DMA chunk, then
    # [128 partitions, <contiguous per-partition payload>]
    xT_d = nc.declare_dram_parameter("xT", [NHALF, 2, 128, KC, N], BF16, isOutput=False)
    af_d = nc.declare_dram_parameter("af", [128, 4, 2, D], FP8, isOutput=False)
    ind_d = nc.declare_dram_parameter("ind", [128, 2, 4, B], FP8, isOutput=False)
    id4_d = nc.declare_dram_parameter("id4", [B, B], F32, isOutput=False)
    w1_d = nc.declare_dram_parameter("w1", [NDFB, 128, KC, DFB], BF16, isOutput=False)
    wa_d = nc.declare_dram_parameter("wa", [NDFB, 128, 4, 2, DFB], FP8, isOutput=False)
    w2_d = nc.declare_dram_parameter("w2", [NDFB, 128, DFB // 128, D], BF16, isOutput=False)
    b1t_d = nc.declare_dram_parameter("b1t", [128, DFF // 128], F32, isOutput=False)
    b2b_d = nc.declare_dram_parameter("b2b", [128, D], F32, isOutput=False)
    out_d = nc.declare_dram_parameter("out", [TOK, D], F32, isOutput=True)

    with tile.TileContext(nc) as tc, ExitStack() as ctx:
        sb = ctx.enter_context(tc.tile_pool(name="sb", bufs=1))
        ps = ctx.enter_context(
            tc.tile_pool(name="ps", bufs=1, space=bass.MemorySpace.PSUM))

        # ---- small persistent tiles -------------------------------------
        ind_t = sb.tile([128, 2, 4, B], FP8, name="ind_t")
        id4_t = sb.tile([B, B], F32, name="id4_t")
        b1t_t = sb.tile([128, DFF // 128], F32, name="b1t_t")
        b2b_t = sb.tile([128, D], F32, name="b2b_t")
        apT_t = sb.tile([128, 2, 4, B], FP8, name="apT_t")
        baud_t = sb.tile([128, DFF // 128, B], F32, name="baud_t")
        junk_t = sb.tile([128, 512], BF16, name="junk_t")
        nc.vector.memset(junk_t[:], 0.5)
        nc.sync.dma_start(out=ind_t[:], in_=ind_d.ap())

        # ---- DMA helpers (weights on the sync queue, in program order) --
        def dma_w1(half, blk):
            w1_t = sb.tile([128, KC, DFB], BF16, name=f"w1_{half}_{blk}",
                           tag="w1s", bufs=2)
            nc.sync.dma_start(out=w1_t[:], in_=w1_d.ap()[blk])
            return w1_t

        def dma_w2(half, blk):
            w2_t = sb.tile([128, DFB // 128, D], BF16,
                           name=f"w2_{half}_{blk}", tag="w2s", bufs=2)
            nc.sync.dma_start(out=w2_t[:], in_=w2_d.ap()[blk])
            return w2_t

        def dma_wa(blk):
            wa_t = sb.tile([128, 4, 2, DFB], FP8, name=f"wa_{blk}",
                           tag="was", bufs=2)
            nc.sync.dma_start(out=wa_t[:], in_=wa_d.ap()[blk])
            return wa_t

        def dma_xT(half, b, xT_t):
            nc.scalar.dma_start(
                out=xT_t[:, b], in_=xT_d.ap()[half, b])

        # ---- start-up: hand-ordered DMA queues --------------------------
        af_t = sb.tile([128, 4, 2, D], FP8, name="af_t")
        nc.sync.dma_start(out=af_t[:], in_=af_d.ap())
        w1_00 = dma_w1(0, 0)
        wa_0 = dma_wa(0)
        w2_00 = dma_w2(0, 0)
        nc.sync.dma_start(out=id4_t[:], in_=id4_d.ap())
        nc.sync.dma_start(out=b1t_t[:], in_=b1t_d.ap())
        nc.sync.dma_start(out=b2b_t[:], in_=b2b_d.ap())
        xT0_t = sb.tile([128, 2, KC, N], BF16, name="xT_0", tag="xT", bufs=2)
        dma_xT(0, 0, xT0_t)
        dma_xT(0, 1, xT0_t)
        xT1_t = sb.tile([128, 2, KC, N], BF16, name="xT_1", tag="xT", bufs=2)
        dma_xT(1, 0, xT1_t)
        dma_xT(1, 1, xT1_t)

        # ---- PE warm-up -------------------------------------------------
        # Bridge the PE from the end of the preamble until af lands (~13us)
        # so the HAM clock gate reaches 8/8 and never re-throttles; a second
        # burst after phase A bridges until the first weight blocks land.
        psW = ps.tile([128, 512], F32, name="psW", tag="ps2b", bufs=2)
        for i in range(13):
            nc.tensor.matmul(psW[:], junk_t[:, 0:128], junk_t[:],
                             start=True, stop=True)

        # ---- phase A: audio mean-pool -> apT [d-chunk, b] ---------------
        # pooled [4, d] = ind.T @ af via fp8 DoubleRow (contraction 256 per
        # matmul), then transpose chunks and re-pack fp8 for phase B.
        ap_sb = sb.tile([B, D], F32, name="ap_sb")
        for dh in range(2):
            psP = ps.tile([B, 512], F32, name=f"psP{dh}",
                          tag=f"ps1{'ab'[dh]}", bufs=2)
            for tc_ in range(4):
                nc.tensor.matmul(
                    psP[:], ind_t[:, :, tc_, :],
                    af_t[:, tc_, :, dh * 512:(dh + 1) * 512],
                    start=(tc_ == 0), stop=(tc_ == 3), perf_mode=DR)
            nc.vector.tensor_copy(ap_sb[:, dh * 512:(dh + 1) * 512], psP[:])
        for dc in range(KC):
            psQ = ps.tile([128, B], F32, name=f"psQ{dc}", tag="ps2a", bufs=2)
            nc.tensor.transpose(
                psQ[:], ap_sb[:, dc * 128:(dc + 1) * 128], id4_t[:])
            nc.vector.tensor_copy(apT_t[:, dc % 2, dc // 2, :], psQ[:])
        for i in range(12):
            nc.tensor.matmul(psW[:], junk_t[:, 0:128], junk_t[:],
                             start=True, stop=True)

        # ---- phase B: main loop -----------------------------------------
        for half in range(NHALF):
            xT_t = xT0_t if half == 0 else xT1_t
            oacc = [
                sb.tile([128, D], F32, name=f"oacc_{half}_{t}",
                        tag=f"oacc{t}", bufs=1)
                for t in range(TOKH // 128)
            ]
            blocks = list(range(NDFB)) if half == 0 else \
                list(reversed(range(NDFB)))
            for blk_i, blk in enumerate(blocks):
                boff = blk * DFB
                first = (half == 0 and blk_i == 0)
                first_blk = (blk_i == 0)
                last_blk = (blk_i == len(blocks) - 1)
                reuse = (half == 1 and blk_i == 0)
                ncc = DFB // 128
                w1_t = (w1_00 if first else
                        (prev_w1 if reuse else dma_w1(half, blk)))
                if half == 0:
                    wa_t = wa_0 if first else dma_wa(blk)
                    # audio_h [4, 512] = apT.T @ wa_block, fp8 DoubleRow
                    psH = ps.tile([B, DFB], F32, name=f"psH{blk}",
                                  tag="ps1a", bufs=2)
                    for dcp in range(4):
                        nc.tensor.matmul(
                            psH[:], apT_t[:, :, dcp, :], wa_t[:, dcp, :, :],
                            start=(dcp == 0), stop=(dcp == 3), perf_mode=DR)
                    ah_t = sb.tile([B, DFB], F32, name=f"ah_{blk}", tag="ah",
                                   bufs=2)
                    nc.vector.tensor_copy(ah_t[:], psH[:])
                    for c in range(ncc):
                        cg = boff // 128 + c
                        psT = ps.tile([128, B], F32, name=f"psT{cg}",
                                      tag="ps2a", bufs=2)
                        nc.tensor.transpose(
                            psT[:], ah_t[:, c * 128:(c + 1) * 128], id4_t[:])
                        nc.vector.tensor_scalar_add(
                            baud_t[:, cg, :], psT[:], b1t_t[:, cg:cg + 1])

                w2_t = (w2_00 if first else
                        (prev_w2 if reuse else dma_w2(half, blk)))
                prev_w1, prev_w2 = w1_t, w2_t

                # GEMM1: h^T tiles [128 dff, 512 tok] for both b-blocks
                hT = []
                for c in range(ncc):
                    cg = boff // 128 + c
                    p1 = [
                        ps.tile([128, N], F32, name=f"ps1_{half}_{blk}_{c}_{b}",
                                tag=f"ps1{'ab'[b]}", bufs=2)
                        for b in range(2)
                    ]
                    for kc in range(KC):
                        for b in range(2):
                            nc.tensor.matmul(
                                p1[b][:], w1_t[:, kc, c * 128:(c + 1) * 128],
                                xT_t[:, b, kc, :],
                                start=(kc == 0), stop=(kc == KC - 1))
                    row = []
                    for b in range(2):
                        bg = half * 2 + b
                        h = sb.tile([128, N], BF16,
                                    name=f"hT_{half}_{blk}_{c}_{b}",
                                    tag=f"hT{c}b{b}", bufs=1)
                        nc.scalar.activation(
                            h[:], p1[b][:], AF.Gelu_apprx_tanh,
                            bias=baud_t[:, cg, bg:bg + 1], scale=1.0)
                        row.append(h)
                    hT.append(row)

                # GEMM2: out tiles [128 tok, 512 d], accumulate over blocks
                for b in range(2):
                    for ts in range(N // 128):
                        tsg = b * (N // 128) + ts
                        p2 = [
                            ps.tile([128, 512], F32,
                                    name=f"ps2_{half}_{blk}_{tsg}_{dh}",
                                    tag=f"ps2{'ab'[dh]}", bufs=2)
                            for dh in range(2)
                        ]
                        for c in range(ncc):
                            for dh in range(2):
                                nc.tensor.matmul(
                                    p2[dh][:],
                                    hT[c][b][:, ts * 128:(ts + 1) * 128],
                                    w2_t[:, c, dh * 512:(dh + 1) * 512],
                                    start=(c == 0), stop=(c == ncc - 1))
                        for dh in range(2):
                            dst = oacc[tsg][:, dh * 512:(dh + 1) * 512]
                            if first_blk:
                                nc.vector.tensor_add(
                                    dst, p2[dh][:],
                                    b2b_t[:, dh * 512:(dh + 1) * 512])
                            else:
                                nc.vector.tensor_add(dst, dst, p2[dh][:])
                            if last_blk:
                                row0 = half * TOKH + tsg * 128
                                nc.sync.dma_start(
                                    out=out_d.ap()[row0:row0 + 128,
                                                   dh * 512:(dh + 1) * 512],
                                    in_=dst)

    nc.compile()
    return nc


def _get_nc():
    if "nc" not in _cache:
        _cache["nc"] = _build()
    return _cache["nc"]


def build_in_maps(x, audio_feat, W1, b1, Wa, W2, b2):
    FP8NP = ml_dtypes.float8_e4m3
    # audio token row t = ki + 128*ko + 256*tc  (tc == batch, NA=256)
    af = np.ascontiguousarray(
        audio_feat.reshape(4, 2, 128, D).transpose(2, 0, 1, 3)
    ).astype(FP8NP)
    ind = np.zeros((128, 2, 4, B), dtype=FP8NP)
    for tc_ in range(4):
        ind[:, :, tc_, tc_] = 1.0 / NA
    id4 = np.eye(B, dtype=np.float32)

    in_maps = []
    for e in range(E):
        xT = np.ascontiguousarray(
            x[:, e].reshape(TOK, D).T
            .reshape(KC, 128, NHALF, 2, N).transpose(2, 3, 1, 0, 4)
        ).astype(ml_dtypes.bfloat16)
        w1p = np.ascontiguousarray(
            W1[e].reshape(KC, 128, NDFB, DFB).transpose(2, 1, 0, 3)
        ).astype(ml_dtypes.bfloat16)
        # wa DoubleRow layout: d = ki + 128*ko + 256*dcp, per dff block
        wap = np.ascontiguousarray(
            Wa[e].reshape(4, 2, 128, NDFB, DFB).transpose(3, 2, 0, 1, 4)
        ).astype(FP8NP)
        w2p = np.ascontiguousarray(
            W2[e].reshape(NDFB, DFB // 128, 128, D).transpose(0, 2, 1, 3)
        ).astype(ml_dtypes.bfloat16)
        in_maps.append({
            "xT": xT,
            "af": af,
            "ind": ind,
            "id4": id4,
            "w1": w1p,
            "wa": wap,
            "w2": w2p,
            "b1t": np.ascontiguousarray(b1[e].reshape(DFF // 128, 128).T),
            "b2b": np.ascontiguousarray(np.broadcast_to(b2[e], (128, D))),
        })
    return in_maps


def kernel(x, audio_feat, W1, b1, Wa, W2, b2):
    x = np.asarray(x, dtype=np.float32)
    audio_feat = np.asarray(audio_feat, dtype=np.float32)
    W1 = np.asarray(W1, dtype=np.float32)
    b1 = np.asarray(b1, dtype=np.float32)
    Wa = np.asarray(Wa, dtype=np.float32)
    W2 = np.asarray(W2, dtype=np.float32)
    b2 = np.asarray(b2, dtype=np.float32)

    nc = _get_nc()
    in_maps = build_in_maps(x, audio_feat, W1, b1, Wa, W2, b2)

    # A prior tenant can leave the accelerator in an unrecoverable state
    # that clears after one failed attempt; retry to absorb that.
    last_err = None
    for attempt in range(3):
        try:
            res = run_bass_kernel_spmd(nc, in_maps, list(range(NC_CORES)))
            break
        except Exception as err:  # noqa: BLE001
            last_err = err
            import time
            time.sleep(2.0)
    else:
        raise last_err

    out = np.empty((B, E, N, D), dtype=np.float32)
    for e in range(E):
        out[:, e] = res.results[e]["out"].reshape(B, N, D)
    return out


# revision 15
# speedup vs baseline: 1.0015x; 1.0015x over previous
"""Expert-parallel MoE FFN kernel for 8 trn2 NeuronCores.

Problem (per full input):
  x [4, 8, 512, 1024], audio_feat [4, 256, 1024],
  W1/Wa [8, 1024, 4096], b1 [8, 4096], W2 [8, 4096, 1024], b2 [8, 1024]
  out[b,e,n,:] = gelu_tanh(x[b,e,n] @ W1[e] + b1[e] + mean(audio_feat[b]) @ Wa[e]) @ W2[e] + b2[e]

Sharding: expert-parallel — core e owns expert e (weights + x[:, e] slice);
audio_feat replicated. No collectives needed: shard/gather on host.

Per-core kernel. Main GEMMs in bf16 (same 1 col/cycle PE rate as fp32r but
~11 ns/matmul faster in practice, half the DMA bytes, and FWL halves the
weight-load); audio path in fp8e4 with DoubleRow perf mode (2 fp8 weights
per PE cell -> half the streaming cycles; audio_h is ~6% of h's magnitude
so fp8 noise is negligible). PSUM accumulation is always fp32.
  - audio pooling via DoubleRow matmul with a block-indicator matrix
  - GEMM1 produces h^T tiles [dff, tok] so GEMM2 can consume them as
    stationary operands without transposes (x passed pre-transposed)
  - dff in 8 blocks of 512; GEMM2 partials accumulate into an SBUF
    accumulator via DVE adds; tokens in 2 halves of 1024.
  - All weight/x tensors are host-packed so every DMA is one contiguous
    run per partition (128 descriptors; ~0.6us HWDGE issue instead of
    3-5us for the strided equivalents).
  - DMA ordering: weights + audio on the sync HWDGE queue in exact
    consumption order (FIFO self-throttles the prefetch); x on the scalar
    queue; output tiles on sync (idle by then). Startup-critical bytes
    (af, first w1/wa/w2 blocks) therefore land first.
  - PE warm-up: ~10 free-dim-512 matmuls over a memset tile warm the HAM
    clock gate to 8/8 while af lands, so the real stream never runs cold.
"""
from contextlib import ExitStack

import ml_dtypes
import numpy as np

import concourse.bass as bass
import concourse.tile as tile
from concourse import bacc, mybir
from concourse.bass_utils import run_bass_kernel_spmd

F32 = mybir.dt.float32
BF16 = mybir.dt.bfloat16
FP8 = mybir.dt.float8e4
AF = mybir.ActivationFunctionType
DR = mybir.MatmulPerfMode.DoubleRow

B, E, N, D = 4, 8, 512, 1024
DFF = 4 * D
NA = 256
TOK = B * N            # 2048 tokens per expert
KC = D // 128          # 8 d-chunks
NHALF = 2              # token halves
TOKH = TOK // NHALF    # 1024
NDFB = 8               # dff blocks
DFB = DFF // NDFB      # 512
NC_CORES = 8

_cache = {}


def _build():
    nc = bacc.Bacc("TRN2", target_bir_lowering=False, debug=False,
                   num_devices=NC_CORES)

    # host-packed layouts: leading dims select the DMA chunk, then
    # [128 partitions, <contiguous per-partition payload>]
    xT_d = nc.declare_dram_parameter("xT", [NHALF, 2, 128, KC, N], BF16, isOutput=False)
    af_d = nc.declare_dram_parameter("af", [128, 4, 2, D], FP8, isOutput=False)
    ind_d = nc.declare_dram_parameter("ind", [128, 2, 4, B], FP8, isOutput=False)
    id4_d = nc.declare_dram_parameter("id4", [B, B], F32, isOutput=False)
    w1_d = nc.declare_dram_parameter("w1", [NDFB, 128, KC, DFB], BF16, isOutput=False)
    wa_d = nc.declare_dram_parameter("wa", [NDFB, 128, 4, 2, DFB], FP8, isOutput=False)
    w2_d = nc.declare_dram_parameter("w2", [NDFB, 128, DFB // 128, D], BF16, isOutput=False)
    b1t_d = nc.declare_dram_parameter("b1t", [128, DFF // 128], F32, isOutput=False)
    b2b_d = nc.declare_dram_parameter("b2b", [128, D], F32, isOutput=False)
    out_d = nc.declare_dram_parameter("out", [TOK, D], F32, isOutput=True)

    with tile.TileContext(nc) as tc, ExitStack() as ctx:
        sb = ctx.enter_context(tc.tile_pool(name="sb", bufs=1))
        ps = ctx.enter_context(
            tc.tile_pool(name="ps", bufs=1, space=bass.MemorySpace.PSUM))

        # ---- small persistent tiles -------------------------------------
        ind_t = sb.tile([128, 2, 4, B], FP8, name="ind_t")
        id4_t = sb.tile([B, B], F32, name="id4_t")
        b1t_t = sb.tile([128, DFF // 128], F32, name="b1t_t")
        b2b_t = sb.tile([128, D], F32, name="b2b_t")
        apT_t = sb.tile([128, 2, 4, B], FP8, name="apT_t")
        baud_t = sb.tile([128, DFF // 128, B], F32, name="baud_t")
        junk_t = sb.tile([128, 512], BF16, name="junk_t")
        nc.vector.memset(junk_t[:], 0.5)
        nc.sync.dma_start(out=ind_t[:], in_=ind_d.ap())

        # ---- DMA helpers (weights on the sync queue, in program order) --
        def dma_w1(half, blk):
            w1_t = sb.tile([128, KC, DFB], BF16, name=f"w1_{half}_{blk}",
                           tag="w1s", bufs=2)
            nc.sync.dma_start(out=w1_t[:], in_=w1_d.ap()[blk])
            return w1_t

        def dma_w2(half, blk):
            w2_t = sb.tile([128, DFB // 128, D], BF16,
                           name=f"w2_{half}_{blk}", tag="w2s", bufs=2)
            nc.sync.dma_start(out=w2_t[:], in_=w2_d.ap()[blk])
            return w2_t

        def dma_wa(blk):
            wa_t = sb.tile([128, 4, 2, DFB], FP8, name=f"wa_{blk}",
                           tag="was", bufs=2)
            nc.sync.dma_start(out=wa_t[:], in_=wa_d.ap()[blk])
            return wa_t

        def dma_xT(half, b, xT_t):
            nc.scalar.dma_start(
                out=xT_t[:, b], in_=xT_d.ap()[half, b])

        # ---- start-up: hand-ordered DMA queues --------------------------
        af_t = sb.tile([128, 4, 2, D], FP8, name="af_t")
        nc.sync.dma_start(out=af_t[:], in_=af_d.ap())
        w1_00 = dma_w1(0, 0)
        wa_0 = dma_wa(0)
        w2_00 = dma_w2(0, 0)
        nc.sync.dma_start(out=id4_t[:], in_=id4_d.ap())
        nc.sync.dma_start(out=b1t_t[:], in_=b1t_d.ap())
        nc.sync.dma_start(out=b2b_t[:], in_=b2b_d.ap())
        xT0_t = sb.tile([128, 2, KC, N], BF16, name="xT_0", tag="xT", bufs=2)
        dma_xT(0, 0, xT0_t)
        dma_xT(0, 1, xT0_t)
        # xT half-1 tile allocated now, DMA'd mid-half-0 (see block loop) so
        # its 2MB don't compete with the startup-critical af/w1/wa/w2 bytes.
        xT1_t = sb.tile([128, 2, KC, N], BF16, name="xT_1", tag="xT", bufs=2)

        # ---- PE warm-up -------------------------------------------------
        # Bridge the PE from the end of the preamble until af lands (~13us)
        # so the HAM clock gate reaches 8/8 and never re-throttles; a second
        # burst after phase A bridges until the first weight blocks land.
        psW = ps.tile([128, 512], F32, name="psW", tag="ps2b", bufs=2)
        for i in range(16):
            nc.tensor.matmul(psW[:], junk_t[:, 0:128], junk_t[:],
                             start=True, stop=True)

        # ---- phase A: audio mean-pool -> apT [d-chunk, b] ---------------
        # pooled [4, d] = ind.T @ af via fp8 DoubleRow (contraction 256 per
        # matmul), then transpose chunks and re-pack fp8 for phase B.
        ap_sb = sb.tile([B, D], F32, name="ap_sb")
        for dh in range(2):
            psP = ps.tile([B, 512], F32, name=f"psP{dh}",
                          tag=f"ps1{'ab'[dh]}", bufs=2)
            for tc_ in range(4):
                nc.tensor.matmul(
                    psP[:], ind_t[:, :, tc_, :],
                    af_t[:, tc_, :, dh * 512:(dh + 1) * 512],
                    start=(tc_ == 0), stop=(tc_ == 3), perf_mode=DR)
            nc.vector.tensor_copy(ap_sb[:, dh * 512:(dh + 1) * 512], psP[:])
        for dc in range(KC):
            psQ = ps.tile([128, B], F32, name=f"psQ{dc}", tag="ps2a", bufs=2)
            nc.tensor.transpose(
                psQ[:], ap_sb[:, dc * 128:(dc + 1) * 128], id4_t[:])
            nc.vector.tensor_copy(apT_t[:, dc % 2, dc // 2, :], psQ[:])
        for i in range(4):
            nc.tensor.matmul(psW[:], junk_t[:, 0:128], junk_t[:],
                             start=True, stop=True)

        # ---- phase B: main loop -----------------------------------------
        for half in range(NHALF):
            xT_t = xT0_t if half == 0 else xT1_t
            oacc = [
                sb.tile([128, D], F32, name=f"oacc_{half}_{t}",
                        tag=f"oacc{t}", bufs=1)
                for t in range(TOKH // 128)
            ]
            blocks = list(range(NDFB)) if half == 0 else \
                list(reversed(range(NDFB)))
            for blk_i, blk in enumerate(blocks):
                if half == 0 and blk_i in (2, 3):
                    dma_xT(1, blk_i - 2, xT1_t)
                boff = blk * DFB
                first = (half == 0 and blk_i == 0)
                first_blk = (blk_i == 0)
                last_blk = (blk_i == len(blocks) - 1)
                reuse = (half == 1 and blk_i == 0)
                ncc = DFB // 128
                w1_t = (w1_00 if first else
                        (prev_w1 if reuse else dma_w1(half, blk)))
                if half == 0:
                    wa_t = wa_0 if first else dma_wa(blk)
                    # audio_h [4, 512] = apT.T @ wa_block, fp8 DoubleRow
                    psH = ps.tile([B, DFB], F32, name=f"psH{blk}",
                                  tag="ps1a", bufs=2)
                    for dcp in range(4):
                        nc.tensor.matmul(
                            psH[:], apT_t[:, :, dcp, :], wa_t[:, dcp, :, :],
                            start=(dcp == 0), stop=(dcp == 3), perf_mode=DR)
                    ah_t = sb.tile([B, DFB], F32, name=f"ah_{blk}", tag="ah",
                                   bufs=2)
                    nc.vector.tensor_copy(ah_t[:], psH[:])
                    for c in range(ncc):
                        cg = boff // 128 + c
                        psT = ps.tile([128, B], F32, name=f"psT{cg}",
                                      tag="ps2a", bufs=2)
                        nc.tensor.transpose(
                            psT[:], ah_t[:, c * 128:(c + 1) * 128], id4_t[:])
                        nc.vector.tensor_scalar_add(
                            baud_t[:, cg, :], psT[:], b1t_t[:, cg:cg + 1])

                w2_t = (w2_00 if first else
                        (prev_w2 if reuse else dma_w2(half, blk)))
                prev_w1, prev_w2 = w1_t, w2_t

                # GEMM1: h^T tiles [128 dff, 512 tok] for both b-blocks
                hT = []
                for c in range(ncc):
                    cg = boff // 128 + c
                    p1 = [
                        ps.tile([128, N], F32, name=f"ps1_{half}_{blk}_{c}_{b}",
                                tag=f"ps1{'ab'[b]}", bufs=2)
                        for b in range(2)
                    ]
                    for kc in range(KC):
                        for b in range(2):
                            nc.tensor.matmul(
                                p1[b][:], w1_t[:, kc, c * 128:(c + 1) * 128],
                                xT_t[:, b, kc, :],
                                start=(kc == 0), stop=(kc == KC - 1))
                    row = []
                    for b in range(2):
                        bg = half * 2 + b
                        h = sb.tile([128, N], BF16,
                                    name=f"hT_{half}_{blk}_{c}_{b}",
                                    tag=f"hT{c}b{b}", bufs=1)
                        nc.scalar.activation(
                            h[:], p1[b][:], AF.Gelu_apprx_tanh,
                            bias=baud_t[:, cg, bg:bg + 1], scale=1.0)
                        row.append(h)
                    hT.append(row)

                # GEMM2: out tiles [128 tok, 512 d], accumulate over blocks
                for b in range(2):
                    for ts in range(N // 128):
                        tsg = b * (N // 128) + ts
                        p2 = [
                            ps.tile([128, 512], F32,
                                    name=f"ps2_{half}_{blk}_{tsg}_{dh}",
                                    tag=f"ps2{'ab'[dh]}", bufs=2)
                            for dh in range(2)
                        ]
                        if last_blk:
                            # dh-major: dh0's accumulation completes while
                            # dh1's matmuls run, so its add + store overlap
                            # them — shortens the end-of-kernel chain.
                            for dh in range(2):
                                for c in range(ncc):
                                    nc.tensor.matmul(
                                        p2[dh][:],
                                        hT[c][b][:, ts * 128:(ts + 1) * 128],
                                        w2_t[:, c, dh * 512:(dh + 1) * 512],
                                        start=(c == 0), stop=(c == ncc - 1))
                        else:
                            for c in range(ncc):
                                for dh in range(2):
                                    nc.tensor.matmul(
                                        p2[dh][:],
                                        hT[c][b][:, ts * 128:(ts + 1) * 128],
                                        w2_t[:, c, dh * 512:(dh + 1) * 512],
                                        start=(c == 0), stop=(c == ncc - 1))
                        for dh in range(2):
                            dst = oacc[tsg][:, dh * 512:(dh + 1) * 512]
                            if first_blk:
                                nc.vector.tensor_add(
                                    dst, p2[dh][:],
                                    b2b_t[:, dh * 512:(dh + 1) * 512])
                            else:
                                nc.vector.tensor_add(dst, dst, p2[dh][:])
                            if last_blk:
                                row0 = half * TOKH + tsg * 128
                                eng = nc.sync if dh == 0 else nc.scalar
                                eng.dma_start(
                                    out=out_d.ap()[row0:row0 + 128,
                                                   dh * 512:(dh + 1) * 512],
                                    in_=dst)

    nc.compile()
    return nc


def _get_nc():
    if "nc" not in _cache:
        _cache["nc"] = _build()
    return _cache["nc"]


def build_in_maps(x, audio_feat, W1, b1, Wa, W2, b2):
    FP8NP = ml_dtypes.float8_e4m3
    # audio token row t = ki + 128*ko + 256*tc  (tc == batch, NA=256)
    af = np.ascontiguousarray(
        audio_feat.reshape(4, 2, 128, D).transpose(2, 0, 1, 3)
    ).astype(FP8NP)
    ind = np.zeros((128, 2, 4, B), dtype=FP8NP)
    for tc_ in range(4):
        ind[:, :, tc_, tc_] = 1.0 / NA
    id4 = np.eye(B, dtype=np.float32)

    in_maps = []
    for e in range(E):
        xT = np.ascontiguousarray(
            x[:, e].reshape(TOK, D).T
            .reshape(KC, 128, NHALF, 2, N).transpose(2, 3, 1, 0, 4)
        ).astype(ml_dtypes.bfloat16)
        w1p = np.ascontiguousarray(
            W1[e].reshape(KC, 128, NDFB, DFB).transpose(2, 1, 0, 3)
        ).astype(ml_dtypes.bfloat16)
        # wa DoubleRow layout: d = ki + 128*ko + 256*dcp, per dff block
        wap = np.ascontiguousarray(
            Wa[e].reshape(4, 2, 128, NDFB, DFB).transpose(3, 2, 0, 1, 4)
        ).astype(FP8NP)
        w2p = np.ascontiguousarray(
            W2[e].reshape(NDFB, DFB // 128, 128, D).transpose(0, 2, 1, 3)
        ).astype(ml_dtypes.bfloat16)
        in_maps.append({
            "xT": xT,
            "af": af,
            "ind": ind,
            "id4": id4,
            "w1": w1p,
            "wa": wap,
            "w2": w2p,
            "b1t": np.ascontiguousarray(b1[e].reshape(DFF // 128, 128).T),
            "b2b": np.ascontiguousarray(np.broadcast_to(b2[e], (128, D))),
        })
    return in_maps


def kernel(x, audio_feat, W1, b1, Wa, W2, b2):
    x = np.asarray(x, dtype=np.float32)
    audio_feat = np.asarray(audio_feat, dtype=np.float32)
    W1 = np.asarray(W1, dtype=np.float32)
    b1 = np.asarray(b1, dtype=np.float32)
    Wa = np.asarray(Wa, dtype=np.float32)
    W2 = np.asarray(W2, dtype=np.float32)
    b2 = np.asarray(b2, dtype=np.float32)

    nc = _get_nc()
    in_maps = build_in_maps(x, audio_feat, W1, b1, Wa, W2, b2)

    # A prior tenant can leave the accelerator in an unrecoverable state
    # that clears after one failed attempt; retry to absorb that.
    last_err = None
    for attempt in range(3):
        try:
            res = run_bass_kernel_spmd(nc, in_maps, list(range(NC_CORES)))
            break
        except Exception as err:  # noqa: BLE001
            last_err = err
            import time
            time.sleep(2.0)
    else:
        raise last_err

    out = np.empty((B, E, N, D), dtype=np.float32)
    for e in range(E):
        out[:, e] = res.results[e]["out"].reshape(B, N, D)
    return out


# revision 22
# speedup vs baseline: 1.0188x; 1.0173x over previous
"""Expert-parallel MoE FFN kernel for 8 trn2 NeuronCores.

Problem (per full input):
  x [4, 8, 512, 1024], audio_feat [4, 256, 1024],
  W1/Wa [8, 1024, 4096], b1 [8, 4096], W2 [8, 4096, 1024], b2 [8, 1024]
  out[b,e,n,:] = gelu_tanh(x[b,e,n] @ W1[e] + b1[e] + mean(audio_feat[b]) @ Wa[e]) @ W2[e] + b2[e]

Sharding: expert-parallel — core e owns expert e (weights + x[:, e] slice);
audio_feat replicated. No collectives needed: shard/gather on host.

Per-core kernel. Main GEMMs in bf16 (same 1 col/cycle PE rate as fp32r but
~11 ns/matmul faster in practice, half the DMA bytes, and FWL halves the
weight-load); audio path in fp8e4 with DoubleRow perf mode (2 fp8 weights
per PE cell -> half the streaming cycles; audio_h is ~6% of h's magnitude
so fp8 noise is negligible). PSUM accumulation is always fp32.
  - audio pooling via DoubleRow matmul with a block-indicator matrix
  - GEMM1 produces h^T tiles [dff, tok] so GEMM2 can consume them as
    stationary operands without transposes (x passed pre-transposed)
  - dff in 8 blocks of 512; GEMM2 partials accumulate into an SBUF
    accumulator via DVE adds; tokens in 2 halves of 1024.
  - All weight/x tensors are host-packed so every DMA is one contiguous
    run per partition (128 descriptors; ~0.6us HWDGE issue instead of
    3-5us for the strided equivalents).
  - DMA ordering: weights + audio on the sync HWDGE queue in exact
    consumption order (FIFO self-throttles the prefetch); x on the scalar
    queue; output tiles on sync (idle by then). Startup-critical bytes
    (af, first w1/wa/w2 blocks) therefore land first.
  - PE warm-up: ~10 free-dim-512 matmuls over a memset tile warm the HAM
    clock gate to 8/8 while af lands, so the real stream never runs cold.
"""
from contextlib import ExitStack

import ml_dtypes
import numpy as np

import concourse.bass as bass
import concourse.tile as tile
from concourse import bacc, mybir
from concourse.bass_utils import run_bass_kernel_spmd

F32 = mybir.dt.float32
BF16 = mybir.dt.bfloat16
FP8 = mybir.dt.float8e4
AF = mybir.ActivationFunctionType
DR = mybir.MatmulPerfMode.DoubleRow

B, E, N, D = 4, 8, 512, 1024
DFF = 4 * D
NA = 256
TOK = B * N            # 2048 tokens per expert
KC = D // 128          # 8 d-chunks
NHALF = 2              # token halves
TOKH = TOK // NHALF    # 1024
NDFB = 8               # dff blocks
DFB = DFF // NDFB      # 512
NC_CORES = 8

_cache = {}


def _build():
    nc = bacc.Bacc("TRN2", target_bir_lowering=False, debug=False,
                   num_devices=NC_CORES)

    # host-packed layouts: leading dims select the DMA chunk, then
    # [128 partitions, <contiguous per-partition payload>]
    xT_d = nc.declare_dram_parameter("xT", [NHALF, 2, 128, KC, N], BF16, isOutput=False)
    af_d = nc.declare_dram_parameter("af", [128, 4, 2, D], FP8, isOutput=False)
    ind_d = nc.declare_dram_parameter("ind", [128, 2, 4, B], FP8, isOutput=False)
    id4_d = nc.declare_dram_parameter("id4", [B, B], F32, isOutput=False)
    w1_d = nc.declare_dram_parameter("w1", [NDFB, 128, KC, DFB], BF16, isOutput=False)
    wa_d = nc.declare_dram_parameter("wa", [NDFB, 128, 4, 2, DFB], FP8, isOutput=False)
    w2_d = nc.declare_dram_parameter("w2", [NDFB, 128, DFB // 128, D], BF16, isOutput=False)
    b1t_d = nc.declare_dram_parameter("b1t", [128, DFF // 128], F32, isOutput=False)
    b2b_d = nc.declare_dram_parameter("b2b", [128, D], F32, isOutput=False)
    out_d = nc.declare_dram_parameter("out", [TOK, D], F32, isOutput=True)

    with tile.TileContext(nc) as tc, ExitStack() as ctx:
        sb = ctx.enter_context(tc.tile_pool(name="sb", bufs=1))
        ps = ctx.enter_context(
            tc.tile_pool(name="ps", bufs=1, space=bass.MemorySpace.PSUM))

        # ---- small persistent tiles -------------------------------------
        ind_t = sb.tile([128, 2, 4, B], FP8, name="ind_t")
        id4_t = sb.tile([B, B], F32, name="id4_t")
        b1t_t = sb.tile([128, DFF // 128], F32, name="b1t_t")
        b2b_t = sb.tile([128, D], F32, name="b2b_t")
        apT_t = sb.tile([128, 2, 4, B], FP8, name="apT_t")
        baud_t = sb.tile([128, DFF // 128, B], F32, name="baud_t")
        junk_t = sb.tile([128, 512], BF16, name="junk_t")
        nc.vector.memset(junk_t[:], 0.5)
        nc.sync.dma_start(out=ind_t[:], in_=ind_d.ap())

        # ---- DMA helpers (weights on the sync queue, in program order) --
        def dma_w1(half, blk):
            w1_t = sb.tile([128, KC, DFB], BF16, name=f"w1_{half}_{blk}",
                           tag="w1s", bufs=2)
            nc.sync.dma_start(out=w1_t[:], in_=w1_d.ap()[blk])
            return w1_t

        def dma_w2(half, blk):
            w2_t = sb.tile([128, DFB // 128, D], BF16,
                           name=f"w2_{half}_{blk}", tag="w2s", bufs=2)
            nc.sync.dma_start(out=w2_t[:], in_=w2_d.ap()[blk])
            return w2_t

        def dma_wa(blk):
            wa_t = sb.tile([128, 4, 2, DFB], FP8, name=f"wa_{blk}",
                           tag="was", bufs=2)
            nc.sync.dma_start(out=wa_t[:], in_=wa_d.ap()[blk])
            return wa_t

        def dma_xT(half, b, xT_t):
            nc.sync.dma_start(
                out=xT_t[:, b], in_=xT_d.ap()[half, b])

        # ---- start-up: hand-ordered DMA queues --------------------------
        # Single sync queue, strict FIFO = strict priority: af gets the full
        # HBM bandwidth first, then the block-0 weights, then x. The scalar
        # queue stays empty so nothing competes with these bytes.
        af_t = sb.tile([128, 4, 2, D], FP8, name="af_t")
        nc.sync.dma_start(out=af_t[:], in_=af_d.ap())
        w1_00 = dma_w1(0, 0)
        wa_0 = dma_wa(0)
        nc.sync.dma_start(out=id4_t[:], in_=id4_d.ap())
        nc.sync.dma_start(out=b1t_t[:], in_=b1t_d.ap())
        xT0_t = sb.tile([128, 2, KC, N], BF16, name="xT_0", tag="xT", bufs=2)
        dma_xT(0, 0, xT0_t)
        dma_xT(0, 1, xT0_t)
        w2_00 = dma_w2(0, 0)
        nc.sync.dma_start(out=b2b_t[:], in_=b2b_d.ap())
        # xT half-1 tile allocated now, DMA'd mid-half-0 (see block loop,
        # behind blk-2 weights whose WAR deps give real backpressure) so its
        # 2MB don't compete with the startup-critical bytes.
        xT1_t = sb.tile([128, 2, KC, N], BF16, name="xT_1", tag="xT", bufs=2)

        # ---- PE warm-up -------------------------------------------------
        # Bridge the PE from the end of the preamble until af lands (~13us)
        # so the HAM clock gate reaches 8/8 and never re-throttles; a second
        # burst after phase A bridges until the first weight blocks land.
        psW = ps.tile([128, 512], F32, name="psW", tag="ps2b", bufs=2)
        for i in range(12):
            nc.tensor.matmul(psW[:], junk_t[:, 0:128], junk_t[:],
                             start=True, stop=True)

        # ---- phase A: audio mean-pool -> apT [d-chunk, b] ---------------
        # pooled [4, d] = ind.T @ af via fp8 DoubleRow (contraction 256 per
        # matmul), then transpose chunks and re-pack fp8 for phase B.
        ap_sb = sb.tile([B, D], F32, name="ap_sb")
        for dh in range(2):
            psP = ps.tile([B, 512], F32, name=f"psP{dh}",
                          tag=f"ps1{'ab'[dh]}", bufs=2)
            for tc_ in range(4):
                nc.tensor.matmul(
                    psP[:], ind_t[:, :, tc_, :],
                    af_t[:, tc_, :, dh * 512:(dh + 1) * 512],
                    start=(tc_ == 0), stop=(tc_ == 3), perf_mode=DR)
            nc.vector.tensor_copy(ap_sb[:, dh * 512:(dh + 1) * 512], psP[:])
        for dc in range(KC):
            psQ = ps.tile([128, B], F32, name=f"psQ{dc}", tag="ps2a", bufs=2)
            nc.tensor.transpose(
                psQ[:], ap_sb[:, dc * 128:(dc + 1) * 128], id4_t[:])
            nc.vector.tensor_copy(apT_t[:, dc % 2, dc // 2, :], psQ[:])
        for i in range(16):
            nc.tensor.matmul(psW[:], junk_t[:, 0:128], junk_t[:],
                             start=True, stop=True)

        # ---- phase B: main loop -----------------------------------------
        for half in range(NHALF):
            xT_t = xT0_t if half == 0 else xT1_t
            oacc = [
                sb.tile([128, D], F32, name=f"oacc_{half}_{t}",
                        tag=f"oacc{t}", bufs=1)
                for t in range(TOKH // 128)
            ]
            blocks = list(range(NDFB)) if half == 0 else \
                list(reversed(range(NDFB)))
            for blk_i, blk in enumerate(blocks):
                boff = blk * DFB
                first = (half == 0 and blk_i == 0)
                first_blk = (blk_i == 0)
                last_blk = (blk_i == len(blocks) - 1)
                reuse = (half == 1 and blk_i == 0)
                ncc = DFB // 128
                w1_t = (w1_00 if first else
                        (prev_w1 if reuse else dma_w1(half, blk)))
                if half == 0 and blk_i in (2, 3):
                    dma_xT(1, blk_i - 2, xT1_t)
                if half == 0:
                    wa_t = wa_0 if first else dma_wa(blk)
                    # audio_h [4, 512] = apT.T @ wa_block, fp8 DoubleRow
                    psH = ps.tile([B, DFB], F32, name=f"psH{blk}",
                                  tag="ps1a", bufs=2)
                    for dcp in range(4):
                        nc.tensor.matmul(
                            psH[:], apT_t[:, :, dcp, :], wa_t[:, dcp, :, :],
                            start=(dcp == 0), stop=(dcp == 3), perf_mode=DR)
                    ah_t = sb.tile([B, DFB], F32, name=f"ah_{blk}", tag="ah",
                                   bufs=2)
                    nc.vector.tensor_copy(ah_t[:], psH[:])
                    for c in range(ncc):
                        cg = boff // 128 + c
                        psT = ps.tile([128, B], F32, name=f"psT{cg}",
                                      tag="ps2a", bufs=2)
                        nc.tensor.transpose(
                            psT[:], ah_t[:, c * 128:(c + 1) * 128], id4_t[:])
                        nc.vector.tensor_scalar_add(
                            baud_t[:, cg, :], psT[:], b1t_t[:, cg:cg + 1])

                w2_t = (w2_00 if first else
                        (prev_w2 if reuse else dma_w2(half, blk)))
                prev_w1, prev_w2 = w1_t, w2_t

                # GEMM1: h^T tiles [128 dff, 512 tok] for both b-blocks
                hT = []
                for c in range(ncc):
                    cg = boff // 128 + c
                    p1 = [
                        ps.tile([128, N], F32, name=f"ps1_{half}_{blk}_{c}_{b}",
                                tag=f"ps1{'ab'[b]}", bufs=2)
                        for b in range(2)
                    ]
                    if first:
                        # b-major so b0's matmuls can start before xT b1 lands
                        for b in range(2):
                            for kc in range(KC):
                                nc.tensor.matmul(
                                    p1[b][:],
                                    w1_t[:, kc, c * 128:(c + 1) * 128],
                                    xT_t[:, b, kc, :],
                                    start=(kc == 0), stop=(kc == KC - 1))
                    else:
                        for kc in range(KC):
                            for b in range(2):
                                nc.tensor.matmul(
                                    p1[b][:],
                                    w1_t[:, kc, c * 128:(c + 1) * 128],
                                    xT_t[:, b, kc, :],
                                    start=(kc == 0), stop=(kc == KC - 1))
                    row = []
                    for b in range(2):
                        bg = half * 2 + b
                        h = sb.tile([128, N], BF16,
                                    name=f"hT_{half}_{blk}_{c}_{b}",
                                    tag=f"hT{c}b{b}", bufs=1)
                        nc.scalar.activation(
                            h[:], p1[b][:], AF.Gelu_apprx_tanh,
                            bias=baud_t[:, cg, bg:bg + 1], scale=1.0)
                        row.append(h)
                    hT.append(row)

                # GEMM2: out tiles [128 tok, 512 d], accumulate over blocks
                for b in range(2):
                    for ts in range(N // 128):
                        tsg = b * (N // 128) + ts
                        p2 = [
                            ps.tile([128, 512], F32,
                                    name=f"ps2_{half}_{blk}_{tsg}_{dh}",
                                    tag=f"ps2{'ab'[dh]}", bufs=2)
                            for dh in range(2)
                        ]
                        if last_blk:
                            # dh-major: dh0's accumulation completes while
                            # dh1's matmuls run, so its add + store overlap
                            # them — shortens the end-of-kernel chain.
                            for dh in range(2):
                                for c in range(ncc):
                                    nc.tensor.matmul(
                                        p2[dh][:],
                                        hT[c][b][:, ts * 128:(ts + 1) * 128],
                                        w2_t[:, c, dh * 512:(dh + 1) * 512],
                                        start=(c == 0), stop=(c == ncc - 1))
                        else:
                            for c in range(ncc):
                                for dh in range(2):
                                    nc.tensor.matmul(
                                        p2[dh][:],
                                        hT[c][b][:, ts * 128:(ts + 1) * 128],
                                        w2_t[:, c, dh * 512:(dh + 1) * 512],
                                        start=(c == 0), stop=(c == ncc - 1))
                        for dh in range(2):
                            dst = oacc[tsg][:, dh * 512:(dh + 1) * 512]
                            if first_blk:
                                nc.vector.tensor_add(
                                    dst, p2[dh][:],
                                    b2b_t[:, dh * 512:(dh + 1) * 512])
                            else:
                                nc.vector.tensor_add(dst, dst, p2[dh][:])
                            if last_blk:
                                row0 = half * TOKH + tsg * 128
                                eng = nc.sync if dh == 0 else nc.scalar
                                eng.dma_start(
                                    out=out_d.ap()[row0:row0 + 128,
                                                   dh * 512:(dh + 1) * 512],
                                    in_=dst)

    nc.compile()
    return nc


def _get_nc():
    if "nc" not in _cache:
        _cache["nc"] = _build()
    return _cache["nc"]


def build_in_maps(x, audio_feat, W1, b1, Wa, W2, b2):
    FP8NP = ml_dtypes.float8_e4m3
    # audio token row t = ki + 128*ko + 256*tc  (tc == batch, NA=256)
    af = np.ascontiguousarray(
        audio_feat.reshape(4, 2, 128, D).transpose(2, 0, 1, 3)
    ).astype(FP8NP)
    ind = np.zeros((128, 2, 4, B), dtype=FP8NP)
    for tc_ in range(4):
        ind[:, :, tc_, tc_] = 1.0 / NA
    id4 = np.eye(B, dtype=np.float32)

    in_maps = []
    for e in range(E):
        xT = np.ascontiguousarray(
            x[:, e].reshape(TOK, D).T
            .reshape(KC, 128, NHALF, 2, N).transpose(2, 3, 1, 0, 4)
        ).astype(ml_dtypes.bfloat16)
        w1p = np.ascontiguousarray(
            W1[e].reshape(KC, 128, NDFB, DFB).transpose(2, 1, 0, 3)
        ).astype(ml_dtypes.bfloat16)
        # wa DoubleRow layout: d = ki + 128*ko + 256*dcp, per dff block
        wap = np.ascontiguousarray(
            Wa[e].reshape(4, 2, 128, NDFB, DFB).transpose(3, 2, 0, 1, 4)
        ).astype(FP8NP)
        w2p = np.ascontiguousarray(
            W2[e].reshape(NDFB, DFB // 128, 128, D).transpose(0, 2, 1, 3)
        ).astype(ml_dtypes.bfloat16)
        in_maps.append({
            "xT": xT,
            "af": af,
            "ind": ind,
            "id4": id4,
            "w1": w1p,
            "wa": wap,
            "w2": w2p,
            "b1t": np.ascontiguousarray(b1[e].reshape(DFF // 128, 128).T),
            "b2b": np.ascontiguousarray(np.broadcast_to(b2[e], (128, D))),
        })
    return in_maps


def kernel(x, audio_feat, W1, b1, Wa, W2, b2):
    x = np.asarray(x, dtype=np.float32)
    audio_feat = np.asarray(audio_feat, dtype=np.float32)
    W1 = np.asarray(W1, dtype=np.float32)
    b1 = np.asarray(b1, dtype=np.float32)
    Wa = np.asarray(Wa, dtype=np.float32)
    W2 = np.asarray(W2, dtype=np.float32)
    b2 = np.asarray(b2, dtype=np.float32)

    nc = _get_nc()
    in_maps = build_in_maps(x, audio_feat, W1, b1, Wa, W2, b2)

    # A prior tenant can leave the accelerator in an unrecoverable state
    # that clears after one failed attempt; retry to absorb that.
    last_err = None
    for attempt in range(3):
        try:
            res = run_bass_kernel_spmd(nc, in_maps, list(range(NC_CORES)))
            break
        except Exception as err:  # noqa: BLE001
            last_err = err
            import time
            time.sleep(2.0)
    else:
        raise last_err

    out = np.empty((B, E, N, D), dtype=np.float32)
    for e in range(E):
        out[:, e] = res.results[e]["out"].reshape(B, N, D)
    return out
